# revision 45
# baseline (speedup 1.0000x reference)
"""Distributed attention kernel for 8 Trainium2 NeuronCores — zero-collective.

reference:
    query = features_host @ Q          # [4096, 1024]
    key   = features_guests @ K        # [8192, 1024]
    value = features_guests @ V        # [8192, 1024]
    att   = softmax(query @ key.T / 32, axis=1)
    out   = att @ value                # [4096, 1024]

Algebraic restructure so each core needs NO cross-core data:
    S   = query @ key^T = (host @ Q @ K^T) @ guests^T = q2 @ guests^T
    out = softmax(S) @ (guests @ V) = (P @ guests) @ V / rowsum = T @ V / rs
Host rows (N=4096) are sharded 512/core; guests (all 8192 rows) are
replicated to every core in two host-prepped layouts (bf16 transposed
for the S sweep, fp8e4 natural pairs for the T sweep). K folds into
the query side (q2 = host Q K^T, 2×1.07 GF) and V applies after guest
aggregation (O = T V, 1.07 GF), so the 16 MB keyT / 16 MB value
all-gathers of the collective formulation disappear entirely — along
with the ~100 us entry barrier and ~230 us of serial AG time.

Per-core pipeline (fp32 PSUM accumulation everywhere, 20.4 GF):
  A: queryT = Q^T-chunks @ hostT; q2T = K @ queryT (bf16). DMAs and
     the accumulation loop are din-slice-granular so the PE starts
     ~1 us after the first 128-row slices of ht/wq land. All
     PSUM->SBUF copy chains alternate ScalarE/VectorE to halve their
     exposure at phase boundaries (PSUM groups hoist deps onto their
     first matmul, so the full chain gates the next phase).
  B: S sweep over 64 m-chunks (bf16 — fp8 here fails the 2e-2 gate:
     quantizing the q2/guest operands puts ~2.6% on the attention
     weights): S^T = guestsT_blk.T @ q2T, exp on ScalarE (scale=1/32)
     written DIRECTLY as fp8e4 into persistent P [m_i, mo2, i, n]
     pair layout; VectorE accumulates the rowsum as each half lands.
     The last S iterations prefetch the first T-sweep chunks and wv.
  C: T^T sweep in fp8e4 DoubleRow (2 elem/cycle, 256-deep
     contraction, measured 1.74% rel err vs the 2e-2 gate):
     T^T[din, n] += guests_pair.T @ P[:, mo2, :, :] into all 8 PSUM
     banks over 32 pair-chunks, consumed in order [1..31, 0] so the
     first group never waits on the S-sweep tail, with the last two
     groups interleaved so the per-bank stops stagger wider than the
     copy-out chains. Off the PE path, gpsimd partition_all_reduce
     collapses the rowsum, DVE 32x32 block transposes of the
     (replicated) rowsum row extract the per-partition layout
     rsT[q] = rowsum[c*128+q] from the diagonal blocks, and a
     reciprocal gives per-n-chunk [128,1] scalars.
  D: copy T^T to SBUF bf16 (ScalarE/VectorE alternating, trailing the
     per-bank accumulation stops); O = T^T-chunks @ V (bf16) with the
     din loop innermost so each output bank completes early; the
     final PSUM->SBUF copy applies the softmax division as a
     per-partition tensor_scalar multiply, then DMA out.

Measured on 8xtrn2: 231.5 us (baseline all-gather formulation:
466 us); PE busy ~208 us at 2.4 GHz with ~4 us of gaps.
"""

import sys

for _p in ("/opt/trn_rl_repo", "/root/.axon_site/_ro/trn_rl_repo"):
    if _p not in sys.path:
        sys.path.insert(0, _p)

import numpy as np

N_HOST = 4096
N_GUEST = 8192
DIM = 1024
N_CORES = 8
N_SH = N_HOST // N_CORES      # 512 host rows per core
P = 128
NMO = N_GUEST // P            # 64 m-chunks of 128

_CACHE = {}


def _build():
    import concourse.bass as bass  # noqa: F401
    import concourse.mybir as mybir
    import concourse.tile as tile
    from concourse import bacc
    import concourse.bass_isa as bass_isa

    f32 = mybir.dt.float32
    bf16 = mybir.dt.bfloat16
    AF = mybir.ActivationFunctionType

    nc = bacc.Bacc(
        "TRN2",
        target_bir_lowering=False,
        debug=False,
        num_devices=N_CORES,
    )

    # host-prepped layouts (see kernel()):
    #   ht  [128, 8, 512]  = host_slice^T as [din_i, din_o, n]
    #   gt  [64, 128, 8, 128] = guests^T chunks [mo][din_i, din_o, m]
    #   gn  [8192, 1024]   = guests natural (bf16)
    #   wq  [128, 8, 1024] = Q as [din_i, din_o, dout]
    #   wkt [128, 8, 1024] = K^T as [dout_i, dout_o, din]
    #   wv  [128, 8, 1024] = V as [din_i, din_o, dout]
    fp8 = mybir.dt.float8e4
    NFP8 = 12   # last 12 S-sweep m-chunks run in fp8 DoubleRow (sim 1.879%)
    ht = nc.dram_tensor("ht", [P, 8, N_SH], bf16, kind="ExternalInput").ap()
    gt = nc.dram_tensor("gt", [NMO - NFP8, P, 8, P], bf16,
                        kind="ExternalInput").ap()
    # guests^T fp8 pair layout for the fp8 S chunks: [c][din_i][j][i][m],
    # logical din = j*256 + i*128 + din_i, m-chunk mo = (NMO-NFP8)+c
    gt8 = nc.dram_tensor("gt8", [NFP8, P, 4, 2, P], fp8,
                         kind="ExternalInput").ap()
    # guests natural in fp8e4 pair layout for DoubleRow: [mo2][m_i][i][din],
    # logical row m = mo2*256 + i*128 + m_i
    gn = nc.dram_tensor("gn", [NMO // 2, P, 2, DIM], fp8,
                        kind="ExternalInput").ap()
    wq = nc.dram_tensor("wq", [P, 8, DIM], bf16, kind="ExternalInput").ap()
    wkt = nc.dram_tensor("wkt", [P, 8, DIM], bf16, kind="ExternalInput").ap()
    wv = nc.dram_tensor("wv", [P, 8, DIM], bf16, kind="ExternalInput").ap()
    out = nc.dram_tensor("out", [N_SH, DIM], f32, kind="ExternalOutput").ap()

    def psum_to_sbuf(idx, dst, src):
        # alternate engines so copy chains run two-wide
        if idx % 2 == 0:
            nc.scalar.copy(out=dst, in_=src)
        else:
            nc.vector.tensor_copy(out=dst, in_=src)

    with tile.TileContext(nc) as tc:
        with tc.tile_pool(name="persist", bufs=1) as persist:
            # P in fp8e4 pair layout [m_i, mo2, i, n] for the DoubleRow
            # T sweep (rhs free dims (2, n)); exp writes one half per mo
            Psb = persist.tile([P, NMO // 2, 2, N_SH], fp8, name="Psb")
            qryT = persist.tile([P, 8, N_SH], bf16, name="qryT")
            q2Ts = [persist.tile([P, N_SH], bf16, name=f"q2T{i}")
                    for i in range(8)]
            ht_sb = persist.tile([P, 8, N_SH], bf16, name="ht_sb")
            wq_sb = persist.tile([P, 8, DIM], bf16, name="wq_sb")
            wkt_sb = persist.tile([P, 8, DIM], bf16, name="wkt_sb")
            wv_sb = persist.tile([P, 8, DIM], bf16, name="wv_sb")
            tTbs = [persist.tile([P, N_SH], bf16, name=f"tTb{i}")
                    for i in range(8)]
            q2T8 = persist.tile([P, 4, 2, N_SH], fp8, name="q2T8")
            rs_acc = persist.tile([P, N_SH], f32, name="rs_acc")
            rs_red = persist.tile([P, N_SH], f32, name="rs_red")
            rs_tr = persist.tile([P, P], f32, name="rs_tr")
            rsT4 = persist.tile([P, 4], f32, name="rsT4")
            recip4 = persist.tile([P, 4], f32, name="recip4")

            # ---- phase A: loads + query/q2 projections (din-sliced) ----
            with tc.tile_pool(name="ps_a", bufs=8, space="PSUM") as ps_a:
                # interleave ht/wq DMA slices so do-group 0 lands first;
                # split wq do=0 so matmul #1 only waits on its dc=0 chunk
                nc.sync.dma_start(ht_sb[:, 0, :], ht[:, 0, :])
                nc.sync.dma_start(wq_sb[:, 0, 0:P], wq[:, 0, 0:P])
                nc.sync.dma_start(wq_sb[:, 0, P:DIM], wq[:, 0, P:DIM])
                for do in range(1, 8):
                    nc.sync.dma_start(ht_sb[:, do, :], ht[:, do, :])
                    nc.sync.dma_start(wq_sb[:, do, :], wq[:, do, :])
                for do in range(8):
                    nc.sync.dma_start(wkt_sb[:, do, :], wkt[:, do, :])

                qp = [ps_a.tile([P, N_SH], f32, name=f"qp{dc}", tag="pa")
                      for dc in range(8)]
                for do in range(8):
                    for dc in range(8):
                        nc.tensor.matmul(
                            qp[dc],
                            lhsT=wq_sb[:, do, dc * P:(dc + 1) * P],
                            rhs=ht_sb[:, do, :],
                            start=(do == 0), stop=(do == 7),
                        )
                for dc in range(8):
                    psum_to_sbuf(dc, qryT[:, dc, :], qp[dc])

                # dc-outer: each q2p bank stops 8 matmuls apart, so its
                # copy-out hides behind the remaining accumulation groups
                q2p = [ps_a.tile([P, N_SH], f32, name=f"q2p{dc}", tag="pa")
                       for dc in range(8)]
                for dc in range(8):
                    for do in range(8):
                        nc.tensor.matmul(
                            q2p[dc],
                            lhsT=wkt_sb[:, do, dc * P:(dc + 1) * P],
                            rhs=qryT[:, do, :],
                            start=(do == 0), stop=(do == 7),
                        )
                    psum_to_sbuf(dc, q2Ts[dc], q2p[dc])

            # fp8 copy of q2T in DoubleRow pair layout for the fp8 S chunks;
            # runs on ACT/DVE during the early (bf16) S sweep
            for do in range(8):
                if do % 2 == 0:
                    nc.scalar.copy(out=q2T8[:, do // 2, do % 2, :],
                                   in_=q2Ts[do])
                else:
                    nc.vector.tensor_copy(out=q2T8[:, do // 2, do % 2, :],
                                          in_=q2Ts[do])

            # ---- phases B+C share the streaming pools ----
            with tc.tile_pool(name="gtp", bufs=1) as gtp, \
                 tc.tile_pool(name="gnp", bufs=1) as gnp:

                NC2 = NMO // 2
                # T-sweep consumption order (see phase C)
                t_order = list(range(1, NC2)) + [0]
                gn_tiles = [None] * NC2

                def issue_gn(k):
                    t_ = gnp.tile([P, 2, DIM], fp8, name="gn_t",
                                  tag="gn", bufs=12)
                    nc.sync.dma_start(t_, gn[k])
                    gn_tiles[k] = t_

                # ---- phase B: S sweep (S^T, exp, rowsum) ----
                with tc.tile_pool(name="ps_st", bufs=4, space="PSUM") as ps_st:
                    for mo in range(NMO):
                        if mo < NMO - NFP8:
                            gt_t = gtp.tile([P, 8, P], bf16, name="gt_t",
                                            tag="gt", bufs=12)
                            nc.sync.dma_start(gt_t, gt[mo])
                        else:
                            g8_t = gtp.tile([P, 4, 2, P], fp8, name="g8_t",
                                            tag="gt8", bufs=4)
                            nc.sync.dma_start(g8_t, gt8[mo - (NMO - NFP8)])
                        if mo == 40:
                            # wv is first needed in phase D
                            nc.sync.dma_start(wv_sb, wv)
                        if mo >= 52:
                            issue_gn(t_order[mo - 52])
                        st = ps_st.tile([P, N_SH], f32, name="st", tag="st")
                        if mo < NMO - NFP8:
                            for do in range(8):
                                nc.tensor.matmul(
                                    st,
                                    lhsT=gt_t[:, do, :],
                                    rhs=q2Ts[do],
                                    start=(do == 0), stop=(do == 7),
                                )
                        else:
                            for j in range(4):
                                nc.tensor.matmul(
                                    st,
                                    lhsT=g8_t[:, j, :, :],
                                    rhs=q2T8[:, j, :, :],
                                    start=(j == 0), stop=(j == 3),
                                    perf_mode=mybir.MatmulPerfMode.DoubleRow,
                                )
                        p_half = Psb[:, mo // 2, mo % 2, :]
                        nc.scalar.activation(
                            p_half, st, AF.Exp, scale=1.0 / 32.0)
                        if mo == 0:
                            nc.vector.tensor_copy(out=rs_acc, in_=p_half)
                        else:
                            nc.vector.tensor_add(
                                out=rs_acc, in0=rs_acc, in1=p_half)

                # rowsum -> per-partition [128, 1] reciprocal scalars, all on
                # gpsimd/DVE during the T sweep, off the PE critical path:
                # partition_all_reduce leaves the full rowsum row in every
                # partition; block-transposing a row-replicated [128,128]
                # chunk makes its diagonal 32x32 blocks column-replicated,
                # so rsT4[q, c] = rowsum[c*128+q] via 4 tiny copies each.
                nc.gpsimd.partition_all_reduce(
                    rs_red, rs_acc, P, bass_isa.ReduceOp.add)
                for c in range(4):
                    nc.vector.transpose(
                        out=rs_tr, in_=rs_red[:, c * P:(c + 1) * P])
                    for j in range(4):
                        nc.vector.tensor_copy(
                            out=rsT4[32 * j:32 * (j + 1), c:c + 1],
                            in_=rs_tr[32 * j:32 * (j + 1), 32 * j:32 * j + 1])
                nc.vector.reciprocal(recip4, rsT4)

                # ---- phase C: T^T sweep (fp8e4 DoubleRow, 256-deep) ----
                # consume mo2 in order [1..31, 0] so the first group only
                # needs exps that finished early (not the S-sweep tail);
                # interleave the last two groups so the 8 per-bank stops
                # stagger wider than the PSUM->SBUF copy chains
                order = t_order
                with tc.tile_pool(name="ps_t", bufs=8, space="PSUM") as ps_t:
                    tt = [ps_t.tile([P, N_SH], f32, name=f"tt{dc}", tag="tt")
                          for dc in range(8)]

                    def t_group(mo2, idx, dcs):
                        for dc in dcs:
                            nc.tensor.matmul(
                                tt[dc],
                                lhsT=gn_tiles[mo2][:, :, dc * P:(dc + 1) * P],
                                rhs=Psb[:, mo2, :, :],
                                start=(idx == 0), stop=(idx == NC2 - 1),
                                perf_mode=mybir.MatmulPerfMode.DoubleRow,
                            )

                    for idx, mo2 in enumerate(order[:-2]):
                        if idx + 12 < NC2:
                            issue_gn(order[idx + 12])
                        t_group(mo2, idx, range(8))
                    for dc in range(8):
                        t_group(order[-2], NC2 - 2, [dc])
                        t_group(order[-1], NC2 - 1, [dc])
                    # PSUM->SBUF copies trail the per-bank accumulation stops
                    for dc in range(8):
                        psum_to_sbuf(dc, tTbs[dc], tt[dc])

                # ---- phase D: O = T^T-chunks @ V, divide on copy-out ----
                with tc.tile_pool(name="ps_o", bufs=4, space="PSUM") as ps_o, \
                     tc.tile_pool(name="outp", bufs=4) as outp:
                    for c in range(4):
                        for h in range(2):
                            o_t = ps_o.tile([P, N_SH], f32, name="o_t",
                                            tag="o")
                            for dc in range(8):
                                nc.tensor.matmul(
                                    o_t,
                                    lhsT=tTbs[dc][:, c * P:(c + 1) * P],
                                    rhs=wv_sb[:, dc, h * N_SH:(h + 1) * N_SH],
                                    start=(dc == 0), stop=(dc == 7),
                                )
                            ot = outp.tile([P, N_SH], f32, name="ot", tag="ot")
                            # softmax divide fused into the tail copy,
                            # split across ACT and DVE
                            if h == 0:
                                nc.scalar.mul(ot, o_t, recip4[:, c:c + 1])
                            else:
                                nc.vector.tensor_scalar_mul(
                                    ot, o_t, recip4[:, c:c + 1])
                            nc.sync.dma_start(
                                out[c * P:(c + 1) * P, h * N_SH:(h + 1) * N_SH],
                                ot)

    nc.compile()
    return nc


def _get_nc():
    if "nc" not in _CACHE:
        _CACHE["nc"] = _build()
    return _CACHE["nc"]


def _prep_shared(features_guests, Q, K, V):
    """Host-side layout prep shared by all cores (cast + transpose only)."""
    import ml_dtypes
    bf = ml_dtypes.bfloat16

    NFP8 = 12
    graw = np.ascontiguousarray(np.asarray(features_guests, dtype=np.float32))
    g = graw.astype(bf)
    # gt[mo, p, do, j] = guests^T[do*128+p, mo*128+j] = g[mo*128+j, do*128+p]
    # (bf16 chunks only — the last NFP8 chunks ship as gt8 instead)
    gt = np.ascontiguousarray(
        g.reshape(NMO, P, 8, P).transpose(0, 3, 2, 1)[:NMO - NFP8])
    g8 = np.clip(graw, -240.0, 240.0).astype(ml_dtypes.float8_e4m3fn)
    # gt8[c, p, j, i, m] = fp8 guests^T[j*256+i*128+p, (NMO-NFP8+c)*128+m]
    gt8 = np.ascontiguousarray(
        g8[(NMO - NFP8) * P:].reshape(NFP8, P, 4, 2, P)
        .transpose(0, 4, 2, 3, 1))
    # gn: fp8e4 pair layout [mo2, m_i, i, din] (TRN e4m3 matches OCP within
    # +-240; guests are ~N(0,1) so the clip never engages)
    gn = np.ascontiguousarray(
        g8.reshape(NMO // 2, 2, P, DIM).transpose(0, 2, 1, 3))

    Qn = np.asarray(Q, dtype=np.float32)
    Kn = np.asarray(K, dtype=np.float32)
    Vn = np.asarray(V, dtype=np.float32)
    wq = np.ascontiguousarray(
        Qn.astype(bf).reshape(8, P, DIM).transpose(1, 0, 2))
    wkt = np.ascontiguousarray(
        Kn.T.astype(bf).reshape(8, P, DIM).transpose(1, 0, 2))
    wv = np.ascontiguousarray(
        Vn.astype(bf).reshape(8, P, DIM).transpose(1, 0, 2))
    return gt, gt8, gn, wq, wkt, wv


def make_in_maps(features_host, features_guests, Q, K, V):
    import ml_dtypes
    bf = ml_dtypes.bfloat16

    gt, gt8, gn, wq, wkt, wv = _prep_shared(features_guests, Q, K, V)
    fh = np.asarray(features_host, dtype=np.float32)

    in_maps = []
    for c in range(N_CORES):
        hs = fh[c * N_SH:(c + 1) * N_SH]           # [512, 1024]
        # ht[p, do, n] = hs[n, do*128+p]
        ht = np.ascontiguousarray(
            hs.T.astype(bf).reshape(8, P, N_SH).transpose(1, 0, 2))
        in_maps.append({
            "ht": ht, "gt": gt, "gt8": gt8, "gn": gn,
            "wq": wq, "wkt": wkt, "wv": wv,
        })
    return in_maps


def kernel(features_host, features_guests, Q, K, V):
    from concourse.bass_utils import run_bass_kernel_spmd

    nc = _get_nc()
    in_maps = make_in_maps(features_host, features_guests, Q, K, V)
    res = run_bass_kernel_spmd(nc, in_maps, core_ids=list(range(N_CORES)))
    outs = [np.asarray(res.results[c]["out"]) for c in range(N_CORES)]
    return np.concatenate(outs, axis=0).astype(np.float32)


# revision 46
# speedup vs baseline: 1.2074x; 1.2074x over previous
"""Distributed attention kernel for 8 Trainium2 NeuronCores — zero-collective.

reference:
    query = features_host @ Q          # [4096, 1024]
    key   = features_guests @ K        # [8192, 1024]
    value = features_guests @ V        # [8192, 1024]
    att   = softmax(query @ key.T / 32, axis=1)
    out   = att @ value                # [4096, 1024]

Algebraic restructure so each core needs NO cross-core data:
    S   = query @ key^T = (host @ Q @ K^T) @ guests^T = q2 @ guests^T
    out = softmax(S) @ (guests @ V) = (P @ guests) @ V / rowsum = T @ V / rs
Host rows (N=4096) are sharded 512/core; guests (all 8192 rows) are
replicated to every core in two host-prepped layouts (bf16 transposed
for the S sweep, fp8e4 natural pairs for the T sweep). K folds into
the query side (q2 = host Q K^T, 2×1.07 GF) and V applies after guest
aggregation (O = T V, 1.07 GF), so the 16 MB keyT / 16 MB value
all-gathers of the collective formulation disappear entirely — along
with the ~100 us entry barrier and ~230 us of serial AG time.

Per-core pipeline (fp32 PSUM accumulation everywhere, 20.4 GF):
  A: queryT = Q^T-chunks @ hostT; q2T = K @ queryT (bf16). DMAs and
     the accumulation loop are din-slice-granular so the PE starts
     ~1 us after the first 128-row slices of ht/wq land. All
     PSUM->SBUF copy chains alternate ScalarE/VectorE to halve their
     exposure at phase boundaries (PSUM groups hoist deps onto their
     first matmul, so the full chain gates the next phase).
  B: S sweep over 64 m-chunks (bf16 — fp8 here fails the 2e-2 gate:
     quantizing the q2/guest operands puts ~2.6% on the attention
     weights): S^T = guestsT_blk.T @ q2T, exp on ScalarE (scale=1/32)
     written DIRECTLY as fp8e4 into persistent P [m_i, mo2, i, n]
     pair layout; VectorE accumulates the rowsum as each half lands.
     The last S iterations prefetch the first T-sweep chunks and wv.
  C: T^T sweep in fp8e4 DoubleRow (2 elem/cycle, 256-deep
     contraction, measured 1.74% rel err vs the 2e-2 gate):
     T^T[din, n] += guests_pair.T @ P[:, mo2, :, :] into all 8 PSUM
     banks over 32 pair-chunks, consumed in order [1..31, 0] so the
     first group never waits on the S-sweep tail, with the last two
     groups interleaved so the per-bank stops stagger wider than the
     copy-out chains. Off the PE path, gpsimd partition_all_reduce
     collapses the rowsum, DVE 32x32 block transposes of the
     (replicated) rowsum row extract the per-partition layout
     rsT[q] = rowsum[c*128+q] from the diagonal blocks, and a
     reciprocal gives per-n-chunk [128,1] scalars.
  D: copy T^T to SBUF bf16 (ScalarE/VectorE alternating, trailing the
     per-bank accumulation stops); O = T^T-chunks @ V (bf16) with the
     din loop innermost so each output bank completes early; the
     final PSUM->SBUF copy applies the softmax division as a
     per-partition tensor_scalar multiply, then DMA out.

Measured on 8xtrn2: 231.5 us (baseline all-gather formulation:
466 us); PE busy ~208 us at 2.4 GHz with ~4 us of gaps.
"""

import sys

for _p in ("/opt/trn_rl_repo", "/root/.axon_site/_ro/trn_rl_repo"):
    if _p not in sys.path:
        sys.path.insert(0, _p)

import numpy as np

N_HOST = 4096
N_GUEST = 8192
DIM = 1024
N_CORES = 8
N_SH = N_HOST // N_CORES      # 512 host rows per core
P = 128
NMO = N_GUEST // P            # 64 m-chunks of 128

_CACHE = {}


def _build():
    import concourse.bass as bass  # noqa: F401
    import concourse.mybir as mybir
    import concourse.tile as tile
    from concourse import bacc
    import concourse.bass_isa as bass_isa

    f32 = mybir.dt.float32
    bf16 = mybir.dt.bfloat16
    AF = mybir.ActivationFunctionType

    nc = bacc.Bacc(
        "TRN2",
        target_bir_lowering=False,
        debug=False,
        num_devices=N_CORES,
    )

    # host-prepped layouts (see kernel()):
    #   ht  [128, 8, 512]  = host_slice^T as [din_i, din_o, n]
    #   gt  [64, 128, 8, 128] = guests^T chunks [mo][din_i, din_o, m]
    #   gn  [8192, 1024]   = guests natural (bf16)
    #   wq  [128, 8, 1024] = Q as [din_i, din_o, dout]
    #   wkt [128, 8, 1024] = K^T as [dout_i, dout_o, din]
    #   wv  [128, 8, 1024] = V as [din_i, din_o, dout]
    fp8 = mybir.dt.float8e4
    NFP8 = 12   # last 12 S-sweep m-chunks run in fp8 DoubleRow (sim 1.879%)
    ht = nc.dram_tensor("ht", [P, 8, N_SH], bf16, kind="ExternalInput").ap()
    gt = nc.dram_tensor("gt", [NMO - NFP8, P, 8, P], bf16,
                        kind="ExternalInput").ap()
    # guests^T fp8 pair layout for the fp8 S chunks: [c][din_i][j][i][m],
    # logical din = j*256 + i*128 + din_i, m-chunk mo = (NMO-NFP8)+c
    gt8 = nc.dram_tensor("gt8", [NFP8, P, 4, 2, P], fp8,
                         kind="ExternalInput").ap()
    # guests natural in fp8e4 pair layout for DoubleRow: [mo2][m_i][i][din],
    # logical row m = mo2*256 + i*128 + m_i
    gn = nc.dram_tensor("gn", [NMO // 2, P, 2, DIM], fp8,
                        kind="ExternalInput").ap()
    wq = nc.dram_tensor("wq", [P, 8, DIM], bf16, kind="ExternalInput").ap()
    wkt = nc.dram_tensor("wkt", [P, 8, DIM], bf16, kind="ExternalInput").ap()
    wv = nc.dram_tensor("wv", [P, 8, DIM], bf16, kind="ExternalInput").ap()
    out = nc.dram_tensor("out", [N_SH, DIM], f32, kind="ExternalOutput").ap()

    def psum_to_sbuf(idx, dst, src):
        # alternate engines so copy chains run two-wide
        if idx % 2 == 0:
            nc.scalar.copy(out=dst, in_=src)
        else:
            nc.vector.tensor_copy(out=dst, in_=src)

    with tile.TileContext(nc) as tc:
        with tc.tile_pool(name="persist", bufs=1) as persist:
            # P in fp8e4 pair layout [m_i, mo2, i, n] for the DoubleRow
            # T sweep (rhs free dims (2, n)); exp writes one half per mo
            Psb = persist.tile([P, NMO // 2, 2, N_SH], fp8, name="Psb")
            qryT = persist.tile([P, 8, N_SH], bf16, name="qryT")
            q2Ts = [persist.tile([P, N_SH], bf16, name=f"q2T{i}")
                    for i in range(8)]
            ht_sb = persist.tile([P, 8, N_SH], bf16, name="ht_sb")
            wq_sb = persist.tile([P, 8, DIM], bf16, name="wq_sb")
            wkt_sb = persist.tile([P, 8, DIM], bf16, name="wkt_sb")
            wv_sb = persist.tile([P, 8, DIM], bf16, name="wv_sb")
            tTbs = [persist.tile([P, N_SH], bf16, name=f"tTb{i}")
                    for i in range(8)]
            q2T8 = persist.tile([P, 4, 2, N_SH], fp8, name="q2T8")
            rs_acc = persist.tile([P, N_SH], f32, name="rs_acc")
            rs_red = persist.tile([P, N_SH], f32, name="rs_red")
            rs_tr = persist.tile([P, P], f32, name="rs_tr")
            rsT4 = persist.tile([P, 4], f32, name="rsT4")
            recip4 = persist.tile([P, 4], f32, name="recip4")

            # ---- phase A: loads + query/q2 projections (din-sliced) ----
            with tc.tile_pool(name="ps_a", bufs=8, space="PSUM") as ps_a:
                # interleave ht/wq DMA slices so do-group 0 lands first;
                # split wq do=0 so matmul #1 only waits on its dc=0 chunk
                nc.sync.dma_start(ht_sb[:, 0, :], ht[:, 0, :])
                nc.sync.dma_start(wq_sb[:, 0, 0:P], wq[:, 0, 0:P])
                nc.sync.dma_start(wq_sb[:, 0, P:DIM], wq[:, 0, P:DIM])
                for do in range(1, 8):
                    nc.sync.dma_start(ht_sb[:, do, :], ht[:, do, :])
                    nc.sync.dma_start(wq_sb[:, do, :], wq[:, do, :])
                for do in range(8):
                    nc.sync.dma_start(wkt_sb[:, do, :], wkt[:, do, :])

                qp = [ps_a.tile([P, N_SH], f32, name=f"qp{dc}", tag="pa")
                      for dc in range(8)]
                for do in range(8):
                    for dc in range(8):
                        nc.tensor.matmul(
                            qp[dc],
                            lhsT=wq_sb[:, do, dc * P:(dc + 1) * P],
                            rhs=ht_sb[:, do, :],
                            start=(do == 0), stop=(do == 7),
                        )
                for dc in range(8):
                    psum_to_sbuf(dc, qryT[:, dc, :], qp[dc])

                # dc-outer: each q2p bank stops 8 matmuls apart, so its
                # copy-out hides behind the remaining accumulation groups
                q2p = [ps_a.tile([P, N_SH], f32, name=f"q2p{dc}", tag="pa")
                       for dc in range(8)]
                for dc in range(8):
                    for do in range(8):
                        nc.tensor.matmul(
                            q2p[dc],
                            lhsT=wkt_sb[:, do, dc * P:(dc + 1) * P],
                            rhs=qryT[:, do, :],
                            start=(do == 0), stop=(do == 7),
                        )
                    psum_to_sbuf(dc, q2Ts[dc], q2p[dc])

            # fp8 copy of q2T in DoubleRow pair layout for the fp8 S chunks;
            # runs on ACT/DVE during the early (bf16) S sweep
            for do in range(8):
                if do % 2 == 0:
                    nc.scalar.copy(out=q2T8[:, do // 2, do % 2, :],
                                   in_=q2Ts[do])
                else:
                    nc.vector.tensor_copy(out=q2T8[:, do // 2, do % 2, :],
                                          in_=q2Ts[do])

            # ---- phases B+C share the streaming pools ----
            with tc.tile_pool(name="gtp", bufs=1) as gtp, \
                 tc.tile_pool(name="gnp", bufs=1) as gnp:

                NC2 = NMO // 2
                # T-sweep consumption order (see phase C)
                t_order = list(range(1, NC2)) + [0]
                gn_tiles = [None] * NC2

                def issue_gn(k):
                    t_ = gnp.tile([P, 2, DIM], fp8, name="gn_t",
                                  tag="gn", bufs=12)
                    nc.sync.dma_start(t_, gn[k])
                    gn_tiles[k] = t_

                # ---- phase B: S sweep (S^T, exp, rowsum) ----
                with tc.tile_pool(name="ps_st", bufs=3, space="PSUM") as ps_st:
                    for mo in range(NMO):
                        if mo < NMO - NFP8:
                            gt_t = gtp.tile([P, 8, P], bf16, name="gt_t",
                                            tag="gt", bufs=12)
                            nc.sync.dma_start(gt_t, gt[mo])
                        else:
                            g8_t = gtp.tile([P, 4, 2, P], fp8, name="g8_t",
                                            tag="gt8", bufs=4)
                            nc.sync.dma_start(g8_t, gt8[mo - (NMO - NFP8)])
                        if mo == 40:
                            # wv is first needed in phase D
                            nc.sync.dma_start(wv_sb, wv)
                        if mo >= 52:
                            issue_gn(t_order[mo - 52])
                        st = ps_st.tile([P, N_SH], f32, name="st", tag="st")
                        if mo < NMO - NFP8:
                            for do in range(8):
                                nc.tensor.matmul(
                                    st,
                                    lhsT=gt_t[:, do, :],
                                    rhs=q2Ts[do],
                                    start=(do == 0), stop=(do == 7),
                                )
                        else:
                            for j in range(4):
                                nc.tensor.matmul(
                                    st,
                                    lhsT=g8_t[:, j, :, :],
                                    rhs=q2T8[:, j, :, :],
                                    start=(j == 0), stop=(j == 3),
                                    perf_mode=mybir.MatmulPerfMode.DoubleRow,
                                )
                        p_half = Psb[:, mo // 2, mo % 2, :]
                        nc.scalar.activation(
                            p_half, st, AF.Exp, scale=1.0 / 32.0)
                        if mo == 0:
                            nc.vector.tensor_copy(out=rs_acc, in_=p_half)
                        else:
                            nc.vector.tensor_add(
                                out=rs_acc, in0=rs_acc, in1=p_half)

                # rowsum -> per-partition [128, 1] reciprocal scalars, all on
                # gpsimd/DVE during the T sweep, off the PE critical path:
                # partition_all_reduce leaves the full rowsum row in every
                # partition; block-transposing a row-replicated [128,128]
                # chunk makes its diagonal 32x32 blocks column-replicated,
                # so rsT4[q, c] = rowsum[c*128+q] via 4 tiny copies each.
                nc.gpsimd.partition_all_reduce(
                    rs_red, rs_acc, P, bass_isa.ReduceOp.add)
                for c in range(4):
                    nc.vector.transpose(
                        out=rs_tr, in_=rs_red[:, c * P:(c + 1) * P])
                    for j in range(4):
                        nc.vector.tensor_copy(
                            out=rsT4[32 * j:32 * (j + 1), c:c + 1],
                            in_=rs_tr[32 * j:32 * (j + 1), 32 * j:32 * j + 1])
                nc.vector.reciprocal(recip4, rsT4)

                # ---- phase C: T^T sweep (fp8e4 DoubleRow, 256-deep) ----
                # consume mo2 in order [1..31, 0] so the first group only
                # needs exps that finished early (not the S-sweep tail);
                # interleave the last two groups so the 8 per-bank stops
                # stagger wider than the PSUM->SBUF copy chains
                order = t_order
                with tc.tile_pool(name="ps_t", bufs=8, space="PSUM") as ps_t:
                    tt = [ps_t.tile([P, N_SH], f32, name=f"tt{dc}", tag="tt")
                          for dc in range(8)]

                    def t_group(mo2, idx, dcs):
                        for dc in dcs:
                            nc.tensor.matmul(
                                tt[dc],
                                lhsT=gn_tiles[mo2][:, :, dc * P:(dc + 1) * P],
                                rhs=Psb[:, mo2, :, :],
                                start=(idx == 0), stop=(idx == NC2 - 1),
                                perf_mode=mybir.MatmulPerfMode.DoubleRow,
                            )

                    for idx, mo2 in enumerate(order[:-2]):
                        if idx + 12 < NC2:
                            issue_gn(order[idx + 12])
                        t_group(mo2, idx, range(8))
                    for dc in range(8):
                        t_group(order[-2], NC2 - 2, [dc])
                        t_group(order[-1], NC2 - 1, [dc])
                    # PSUM->SBUF copies trail the per-bank accumulation stops
                    for dc in range(8):
                        psum_to_sbuf(dc, tTbs[dc], tt[dc])

                # ---- phase D: O = T^T-chunks @ V, divide on copy-out ----
                with tc.tile_pool(name="ps_o", bufs=4, space="PSUM") as ps_o, \
                     tc.tile_pool(name="outp", bufs=4) as outp:
                    for c in range(4):
                        for h in range(2):
                            o_t = ps_o.tile([P, N_SH], f32, name="o_t",
                                            tag="o")
                            for dc in range(8):
                                nc.tensor.matmul(
                                    o_t,
                                    lhsT=tTbs[dc][:, c * P:(c + 1) * P],
                                    rhs=wv_sb[:, dc, h * N_SH:(h + 1) * N_SH],
                                    start=(dc == 0), stop=(dc == 7),
                                )
                            ot = outp.tile([P, N_SH], f32, name="ot", tag="ot")
                            # softmax divide fused into the tail copy,
                            # split across ACT and DVE
                            if h == 0:
                                nc.scalar.mul(ot, o_t, recip4[:, c:c + 1])
                            else:
                                nc.vector.tensor_scalar_mul(
                                    ot, o_t, recip4[:, c:c + 1])
                            nc.sync.dma_start(
                                out[c * P:(c + 1) * P, h * N_SH:(h + 1) * N_SH],
                                ot)

    nc.compile()
    return nc


def _get_nc():
    if "nc" not in _CACHE:
        _CACHE["nc"] = _build()
    return _CACHE["nc"]


def _prep_shared(features_guests, Q, K, V):
    """Host-side layout prep shared by all cores (cast + transpose only)."""
    import ml_dtypes
    bf = ml_dtypes.bfloat16

    NFP8 = 12
    graw = np.ascontiguousarray(np.asarray(features_guests, dtype=np.float32))
    g = graw.astype(bf)
    # gt[mo, p, do, j] = guests^T[do*128+p, mo*128+j] = g[mo*128+j, do*128+p]
    # (bf16 chunks only — the last NFP8 chunks ship as gt8 instead)
    gt = np.ascontiguousarray(
        g.reshape(NMO, P, 8, P).transpose(0, 3, 2, 1)[:NMO - NFP8])
    g8 = np.clip(graw, -240.0, 240.0).astype(ml_dtypes.float8_e4m3fn)
    # gt8[c, p, j, i, m] = fp8 guests^T[j*256+i*128+p, (NMO-NFP8+c)*128+m]
    gt8 = np.ascontiguousarray(
        g8[(NMO - NFP8) * P:].reshape(NFP8, P, 4, 2, P)
        .transpose(0, 4, 2, 3, 1))
    # gn: fp8e4 pair layout [mo2, m_i, i, din] (TRN e4m3 matches OCP within
    # +-240; guests are ~N(0,1) so the clip never engages)
    gn = np.ascontiguousarray(
        g8.reshape(NMO // 2, 2, P, DIM).transpose(0, 2, 1, 3))

    Qn = np.asarray(Q, dtype=np.float32)
    Kn = np.asarray(K, dtype=np.float32)
    Vn = np.asarray(V, dtype=np.float32)
    wq = np.ascontiguousarray(
        Qn.astype(bf).reshape(8, P, DIM).transpose(1, 0, 2))
    wkt = np.ascontiguousarray(
        Kn.T.astype(bf).reshape(8, P, DIM).transpose(1, 0, 2))
    wv = np.ascontiguousarray(
        Vn.astype(bf).reshape(8, P, DIM).transpose(1, 0, 2))
    return gt, gt8, gn, wq, wkt, wv


def make_in_maps(features_host, features_guests, Q, K, V):
    import ml_dtypes
    bf = ml_dtypes.bfloat16

    gt, gt8, gn, wq, wkt, wv = _prep_shared(features_guests, Q, K, V)
    fh = np.asarray(features_host, dtype=np.float32)

    in_maps = []
    for c in range(N_CORES):
        hs = fh[c * N_SH:(c + 1) * N_SH]           # [512, 1024]
        # ht[p, do, n] = hs[n, do*128+p]
        ht = np.ascontiguousarray(
            hs.T.astype(bf).reshape(8, P, N_SH).transpose(1, 0, 2))
        in_maps.append({
            "ht": ht, "gt": gt, "gt8": gt8, "gn": gn,
            "wq": wq, "wkt": wkt, "wv": wv,
        })
    return in_maps


def kernel(features_host, features_guests, Q, K, V):
    from concourse.bass_utils import run_bass_kernel_spmd

    nc = _get_nc()
    in_maps = make_in_maps(features_host, features_guests, Q, K, V)
    res = run_bass_kernel_spmd(nc, in_maps, core_ids=list(range(N_CORES)))
    outs = [np.asarray(res.results[c]["out"]) for c in range(N_CORES)]
    return np.concatenate(outs, axis=0).astype(np.float32)
